# revision 14
# baseline (speedup 1.0000x reference)
"""Multi-head attention (16 heads, dk=dv=64) + Gaussian positional prior +
causal mask + softmax + out-proj + residual + LayerNorm, returning
(out [B,S,D] f32, attn [B,H,S,S] f32).

Sharding: data-parallel over batch B=8 across the 8 NeuronCores.

Device dataflow (per core, batch element b):
  - Everything runs "transposed" so each matmul has its contraction dim on
    SBUF partitions:
      QT[hk, s] = (Wq/8).T-as-lhsT @ x.T          (x.T supplied by host)
      KT[hk, s] likewise, V[s, hv] in normal layout (for AV lhsT)
      scoresT[k, q] = KT_h.T-slice @ QT_h-slice   (K=64 contraction)
      gm-band (gauss bias + -1e9 causal mask) added into scores PSUM via an
        identity matmul (rhs = per-head band tile built on ACT from a d^2 table)
      expST = exp(scoresT + gm) on ACT, straight from PSUM -> bf16 SBUF
      AV: lhsT = [V_h | ones] (65 cols) -> ctx rows 0..63, row 64 = rowsums
      attnT[k, q] = expST * (1/rowsum[q]) broadcast (step-0 DMA broadcast)
      fc: out[s, d] = ctxT-as-lhsT @ Wfc.T, + residual, LayerNorm on DVE/ACT
  - attn is written k-major ([H, S(k), S(q)] bf16); the host transposes back
    to [H, S(q), S(k)] f32. The masked upper triangle is never written and
    relies on the run path's zero-initialized output buffers.
"""

import numpy as np
import ml_dtypes

import concourse.bass as bass
import concourse.tile as tile
from concourse import bacc, mybir
from concourse.bass_utils import run_bass_kernel_spmd

B, S, D = 8, 1024, 1024
H, DK, DV = 16, 64, 64
EPS = 1e-5
P = 128
NK = S // P          # 8 k-tiles of 128
NQ = S // 512        # 2 q-chunks of 512
# gm band-tile offsets (off = qc*512 - kt*128); first 4 also carry the mask.
# GM_W[i]: columns [0, w) of the tile that can hold nonzero gauss/mask values;
# outside the window gm == 0 so all gm work (ACT gen, msk add, identity-MM)
# runs on the window slice only.
OFFS = [0, -128, -256, -384, 128]
GM_W = [256, 384, 512, 512, 128]
F32 = mybir.dt.float32
BF16 = mybir.dt.bfloat16
AF = mybir.ActivationFunctionType


def _kts_for_qc(qc):
    # causal [k,q] tiles: tile has any unmasked cell iff kt*128 <= qc*512+511
    return list(range(4)) if qc == 0 else list(range(8))


def _gm_idx(kt, qc):
    off = qc * 512 - kt * 128
    return OFFS.index(off) if off in OFFS else None


def build_program():
    nc = bacc.Bacc("TRN2")

    xT_d = nc.dram_tensor("xT", [D, S], BF16, kind="ExternalInput")
    kT_d = nc.dram_tensor("kT", [D, S], BF16, kind="ExternalInput")
    vT_d = nc.dram_tensor("vT", [D, S], BF16, kind="ExternalInput")
    xres_d = nc.dram_tensor("xres", [S, D], F32, kind="ExternalInput")
    wq_d = nc.dram_tensor("wqT", [D, H * DK], BF16, kind="ExternalInput")
    wk_d = nc.dram_tensor("wkT", [D, H * DK], BF16, kind="ExternalInput")
    wv_d = nc.dram_tensor("wvT", [D, H * DV], BF16, kind="ExternalInput")
    wfc_d = nc.dram_tensor("wfcT", [H * DV, D], BF16, kind="ExternalInput")
    ga_d = nc.dram_tensor("ga", [len(OFFS), P, 512], F32, kind="ExternalInput")
    ms_d = nc.dram_tensor("ms", [4, P, 512], BF16, kind="ExternalInput")
    eye_d = nc.dram_tensor("eye", [P, P], BF16, kind="ExternalInput")

    attn_d = nc.dram_tensor("attnT", [H, S, S], BF16, kind="ExternalOutput")
    out_d = nc.dram_tensor("out", [S, D], F32, kind="ExternalOutput")

    with tile.TileContext(nc) as tc:
        with (
            tc.tile_pool(name="singles", bufs=1) as singles,
            tc.tile_pool(name="wtiles", bufs=2) as wtiles,
            tc.tile_pool(name="proj", bufs=2) as proj,
            tc.tile_pool(name="gmp", bufs=10) as gmp,
            tc.tile_pool(name="expp", bufs=3) as expp,
            tc.tile_pool(name="smallp", bufs=3) as smallp,
            tc.tile_pool(name="lnp", bufs=2) as lnp,
            tc.tile_pool(name="psum", bufs=2, space="PSUM") as psum,
        ):
            # ---------------- phase A: resident loads + V projection ----
            xT_sb = singles.tile([P, NK, S], BF16, tag="xT")
            kT_sb = singles.tile([P, NK, S], BF16, tag="kT")
            wv_sb = singles.tile([P, NK, H * DV], BF16, tag="wv")
            wfc_sb = singles.tile([P, NK, D], BF16, tag="wfc")
            ga_sb = singles.tile([P, len(OFFS), 512], F32, tag="ga")
            ms_sb = singles.tile([P, 4, 512], BF16, tag="ms")
            eye_sb = singles.tile([P, P], BF16, tag="eye")
            eps_sb = singles.tile([P, 1], F32, tag="eps")
            # V with a ones column appended per head: [.., h*65+64] == 1.0
            vaug_sb = singles.tile([P, NK, H * 65], BF16, tag="vaug")

            nc.sync.dma_start(xT_sb, xT_d.ap().rearrange("(o p) s -> p o s", p=P))
            nc.sync.dma_start(kT_sb, kT_d.ap().rearrange("(o p) s -> p o s", p=P))
            nc.sync.dma_start(wv_sb, wv_d.ap().rearrange("(o p) n -> p o n", p=P))
            nc.sync.dma_start(wfc_sb, wfc_d.ap().rearrange("(o p) n -> p o n", p=P))
            nc.sync.dma_start(ga_sb, ga_d.ap().rearrange("o p f -> p o f"))
            nc.sync.dma_start(ms_sb, ms_d.ap().rearrange("o p f -> p o f"))
            nc.sync.dma_start(eye_sb, eye_d.ap())
            nc.vector.memset(eps_sb, EPS)
            vaug4 = vaug_sb.rearrange("p o (h c) -> p o h c", c=65)
            nc.vector.memset(vaug4[:, :, :, 64:65], 1.0)

            # V = input_V @ Wv.T in normal [s, hv] layout
            for st in range(NK):
                ps = [psum.tile([P, 512], F32, tag="proj", name=f"vps{dc}")
                      for dc in range(2)]
                vt = wtiles.tile([P, NK, P], BF16, tag="vt")
                nc.sync.dma_start(
                    vt, vT_d.ap()[:, st * P:(st + 1) * P].rearrange(
                        "(o p) s -> p o s", p=P))
                for k in range(NK):
                    for dc in range(2):
                        nc.tensor.matmul(
                            ps[dc], vt[:, k, :],
                            wv_sb[:, k, dc * 512:(dc + 1) * 512],
                            start=(k == 0), stop=(k == NK - 1))
                for dc in range(2):
                    nc.vector.tensor_copy(
                        out=vaug4[:, st, dc * 8:(dc + 1) * 8, 0:64],
                        in_=ps[dc].rearrange("p (h c) -> p h c", c=64))

            # ctxT accumulator [hv%128, hv//128, q]
            ctxT_sb = singles.tile([P, NK, S], BF16, tag="ctxT")

            # ---------------- phase B: per head-pair ----------------
            for j in range(8):
                hA, hB = 2 * j, 2 * j + 1
                # QT_j / KT_j: [hk%128, s] for hk in [128j, 128j+128)
                qkt = {}
                for nm, w_d, rhs_sb in (("q", wq_d, xT_sb), ("k", wk_d, kT_sb)):
                    dst = proj.tile([P, S], BF16, tag=f"{nm}tj")
                    wt = wtiles.tile([P, NK, P], BF16, tag=f"w{nm}")
                    nc.sync.dma_start(
                        wt, w_d.ap()[:, j * P:(j + 1) * P].rearrange(
                            "(o p) n -> p o n", p=P))
                    for qc in range(2):
                        ps = psum.tile([P, 512], F32, tag="proj")
                        for k in range(NK):
                            nc.tensor.matmul(
                                ps, wt[:, k, :],
                                rhs_sb[:, k, qc * 512:(qc + 1) * 512],
                                start=(k == 0), stop=(k == NK - 1))
                        nc.vector.tensor_copy(
                            out=dst[:, qc * 512:(qc + 1) * 512], in_=ps)
                    qkt[nm] = dst

                # per-head gm band tiles: gauss(+mask)
                gms = {}
                for h in (hA, hB):
                    c_h = 1.0 / (2.0 * float(h + 1) ** 2)
                    for i in range(len(OFFS)):
                        w = GM_W[i]
                        g = gmp.tile([P, w], BF16, tag="gm")
                        nc.scalar.activation(
                            out=g, in_=ga_sb[:, i, :w], func=AF.Exp,
                            scale=-c_h)
                        if i < 4:
                            nc.vector.tensor_add(
                                out=g, in0=g, in1=ms_sb[:, i, :w])
                        gms[(h, i)] = g

                for qc in range(2):
                    kts = _kts_for_qc(qc)
                    exps = {h: expp.tile([P, NK, 512], BF16, tag="expST",
                                         name=f"expST{h % 2}")
                            for h in (hA, hB)}
                    ctxps = {h: psum.tile([65, 512], F32, tag="ctx",
                                          name=f"ctx{h % 2}")
                             for h in (hA, hB)}
                    for kt in kts:
                        gi = _gm_idx(kt, qc)
                        for h in (hA, hB):
                            hp = 64 * (h % 2)
                            sT = psum.tile([P, 512], F32, tag="sT")
                            nc.tensor.matmul(
                                sT,
                                qkt["k"][hp:hp + 64, kt * P:(kt + 1) * P],
                                qkt["q"][hp:hp + 64, qc * 512:(qc + 1) * 512],
                                start=True, stop=(gi is None))
                            if gi is not None:
                                nc.tensor.matmul(
                                    sT[:, :GM_W[gi]], eye_sb, gms[(h, gi)],
                                    start=False, stop=True)
                            nc.scalar.activation(
                                out=exps[h][:, kt, :], in_=sT, func=AF.Exp)
                            nc.tensor.matmul(
                                ctxps[h],
                                vaug_sb[:, kt, h * 65:(h + 1) * 65],
                                exps[h][:, kt, :],
                                start=(kt == kts[0]), stop=(kt == kts[-1]))
                    for h in (hA, hB):
                        hp = 64 * (h % 2)
                        nkt = len(kts)
                        # 1/rowsum via ACT: exp(-ln(sum)); ~1e-5 rel err and
                        # keeps the costly single-partition divide off DVE
                        ln = smallp.tile([1, 512], F32, tag="lnS")
                        nc.scalar.activation(
                            out=ln, in_=ctxps[h][64:65, :], func=AF.Ln)
                        rn = smallp.tile([1, 512], BF16, tag="recipN")
                        nc.scalar.activation(
                            out=rn, in_=ln, func=AF.Exp, scale=-1.0)
                        rb = smallp.tile([P, 512], BF16, tag="recipB")
                        nc.gpsimd.partition_broadcast(rb, rn)
                        nc.vector.tensor_mul(
                            out=ctxT_sb[hp:hp + 64, j, qc * 512:(qc + 1) * 512],
                            in0=ctxps[h][0:64, :], in1=rb[hp:hp + 64, :])
                        rb_b = bass.AP(
                            tensor=rb.tensor, offset=rb.offset,
                            ap=[list(rb.ap[0]), [0, nkt], list(rb.ap[1])])
                        nc.vector.tensor_mul(
                            out=exps[h][:, 0:nkt, :],
                            in0=exps[h][:, 0:nkt, :], in1=rb_b)
                        nc.gpsimd.dma_start(
                            out=attn_d.ap()[h].rearrange(
                                "(kt p) q -> p kt q",
                                p=P)[:, 0:nkt, qc * 512:(qc + 1) * 512],
                            in_=exps[h][:, 0:nkt, :])

            # ---------------- phase C: fc + residual + LayerNorm --------
            for st in range(NK):
                ps = [psum.tile([P, 512], F32, tag="fc", name=f"fcps{dc}")
                      for dc in range(2)]
                for dc in range(2):
                    for jj in range(NK):
                        nc.tensor.matmul(
                            ps[dc],
                            ctxT_sb[:, jj, st * P:(st + 1) * P],
                            wfc_sb[:, jj, dc * 512:(dc + 1) * 512],
                            start=(jj == 0), stop=(jj == NK - 1))
                xr = lnp.tile([P, D], F32, tag="xr")
                nc.sync.dma_start(xr, xres_d.ap()[st * P:(st + 1) * P, :])
                x2 = lnp.tile([P, D], F32, tag="x2")
                for dc in range(2):
                    nc.vector.tensor_add(
                        out=x2[:, dc * 512:(dc + 1) * 512], in0=ps[dc],
                        in1=xr[:, dc * 512:(dc + 1) * 512])
                stats = lnp.tile([P, 2, nc.vector.BN_STATS_DIM], F32, tag="bs")
                for g in range(2):
                    nc.vector.bn_stats(
                        out=stats[:, g, :], in_=x2[:, g * 512:(g + 1) * 512])
                mv = lnp.tile([P, nc.vector.BN_AGGR_DIM], F32, tag="mv")
                nc.vector.bn_aggr(out=mv, in_=stats)
                sd = lnp.tile([P, 1], F32, tag="sd")
                nc.scalar.activation(
                    out=sd, in_=mv[:, 1:2], func=AF.Sqrt, bias=eps_sb)
                nc.vector.reciprocal(out=sd, in_=sd)
                nc.vector.tensor_scalar(
                    out=x2, in0=x2, scalar1=mv[:, 0:1], scalar2=sd,
                    op0=mybir.AluOpType.subtract, op1=mybir.AluOpType.mult)
                nc.sync.dma_start(out=out_d.ap()[st * P:(st + 1) * P, :], in_=x2)

    nc.finalize()
    return nc


_prog_cache = {}


def _get_prog():
    if "nc" not in _prog_cache:
        _prog_cache["nc"] = build_program()
    return _prog_cache["nc"]


def _host_tables():
    ga = np.full((len(OFFS), P, 512), 1e13, dtype=np.float32)
    ms = np.zeros((4, P, 512), dtype=np.float32)
    pp = np.arange(P)[:, None]
    ff = np.arange(512)[None, :]
    for i, off in enumerate(OFFS):
        d = off + ff - pp
        band = (d >= 0) & (d <= 127)
        ga[i][band] = (d * d)[band].astype(np.float32)
        if i < 4:
            ms[i][d < 0] = -1e9
    return ga, ms.astype(ml_dtypes.bfloat16)


def _numpy_fallback(input_Q, input_K, input_V, attn_mask, Wq, Wk, Wv, Wfc):
    # generic-mask reference path (only used if attn_mask is not causal)
    b, s, d = input_Q.shape
    Q = (input_Q @ Wq.T).reshape(b, s, H, DK).transpose(0, 2, 1, 3)
    K = (input_K @ Wk.T).reshape(b, s, H, DK).transpose(0, 2, 1, 3)
    V = (input_V @ Wv.T).reshape(b, s, H, DV).transpose(0, 2, 1, 3)
    scores = np.einsum("bhqd,bhkd->bhqk", Q, K) / np.sqrt(np.float32(DK))
    i = np.arange(s)[:, None]
    jj = np.arange(s)[None, :]
    dist2 = ((i - jj).astype(np.float32)) ** 2
    sigma = np.arange(1, H + 1, dtype=np.float32)
    bias = np.exp(-dist2[None] / (2.0 * sigma[:, None, None] ** 2))
    bias = np.where(i >= jj, bias, 0.0)
    scores = scores + bias[None]
    scores = np.where(attn_mask[:, None], np.float32(-1e9), scores)
    scores -= scores.max(axis=-1, keepdims=True)
    e = np.exp(scores)
    attn = e / e.sum(axis=-1, keepdims=True)
    context = np.einsum("bhqk,bhkd->bhqd", attn, V)
    context = context.transpose(0, 2, 1, 3).reshape(b, s, H * DV)
    output = context @ Wfc.T
    x = output + input_Q
    mu = x.mean(axis=-1, keepdims=True)
    var = x.var(axis=-1, keepdims=True)
    out = (x - mu) / np.sqrt(var + EPS)
    return out.astype(np.float32), attn.astype(np.float32)


def kernel(input_Q, input_K, input_V, attn_mask, Wq, Wk, Wv, Wfc):
    input_Q = np.asarray(input_Q, dtype=np.float32)
    input_K = np.asarray(input_K, dtype=np.float32)
    input_V = np.asarray(input_V, dtype=np.float32)
    attn_mask = np.asarray(attn_mask)
    causal = np.triu(np.ones((S, S), dtype=bool), k=1)
    if not np.array_equal(attn_mask, np.broadcast_to(causal, (B, S, S))):
        return _numpy_fallback(input_Q, input_K, input_V, attn_mask,
                               np.asarray(Wq, np.float32),
                               np.asarray(Wk, np.float32),
                               np.asarray(Wv, np.float32),
                               np.asarray(Wfc, np.float32))

    bf = ml_dtypes.bfloat16
    wqT = (np.asarray(Wq, np.float32).T / np.sqrt(np.float32(DK))).astype(bf)
    wkT = np.asarray(Wk, np.float32).T.astype(bf)
    wvT = np.asarray(Wv, np.float32).T.astype(bf)
    wfcT = np.asarray(Wfc, np.float32).T.astype(bf)
    ga, ms = _host_tables()
    eye = np.eye(P, dtype=np.float32).astype(bf)

    in_maps = []
    for b in range(B):
        in_maps.append({
            "xT": np.ascontiguousarray(input_Q[b].T).astype(bf),
            "kT": np.ascontiguousarray(input_K[b].T).astype(bf),
            "vT": np.ascontiguousarray(input_V[b].T).astype(bf),
            "xres": np.ascontiguousarray(input_Q[b]),
            "wqT": wqT, "wkT": wkT, "wvT": wvT, "wfcT": wfcT,
            "ga": ga, "ms": ms, "eye": eye,
        })

    global _last_in_maps
    _last_in_maps = in_maps
    nc = _get_prog()
    res = run_bass_kernel_spmd(nc, in_maps, core_ids=list(range(B)))

    out = np.empty((B, S, D), dtype=np.float32)
    attn = np.empty((B, H, S, S), dtype=np.float32)
    for b in range(B):
        out[b] = res.results[b]["out"]
        attn[b] = res.results[b]["attnT"].astype(np.float32).transpose(0, 2, 1)
    return out, attn


# revision 15
# speedup vs baseline: 1.0433x; 1.0433x over previous
"""Multi-head attention (16 heads, dk=dv=64) + Gaussian positional prior +
causal mask + softmax + out-proj + residual + LayerNorm, returning
(out [B,S,D] f32, attn [B,H,S,S] f32).

Sharding: data-parallel over batch B=8 across the 8 NeuronCores.

Device dataflow (per core, batch element b):
  - Everything runs "transposed" so each matmul has its contraction dim on
    SBUF partitions:
      QT[hk, s] = (Wq/8).T-as-lhsT @ x.T          (x.T supplied by host)
      KT[hk, s] likewise, V[s, hv] in normal layout (for AV lhsT)
      scoresT[k, q] = KT_h.T-slice @ QT_h-slice   (K=64 contraction)
      gm-band (gauss bias + -1e9 causal mask) added into scores PSUM via an
        identity matmul (rhs = per-head band tile built on ACT from a d^2 table)
      expST = exp(scoresT + gm) on ACT, straight from PSUM -> bf16 SBUF
      AV: lhsT = [V_h | ones] (65 cols) -> ctx rows 0..63, row 64 = rowsums
      attnT[k, q] = expST * (1/rowsum[q]) broadcast (step-0 DMA broadcast)
      fc: out[s, d] = ctxT-as-lhsT @ Wfc.T, + residual, LayerNorm on DVE/ACT
  - attn is written k-major ([H, S(k), S(q)] bf16); the host transposes back
    to [H, S(q), S(k)] f32. The masked upper triangle is never written and
    relies on the run path's zero-initialized output buffers.
"""

import numpy as np
import ml_dtypes

import concourse.bass as bass
import concourse.tile as tile
from concourse import bacc, mybir
from concourse.bass_utils import run_bass_kernel_spmd

B, S, D = 8, 1024, 1024
H, DK, DV = 16, 64, 64
EPS = 1e-5
P = 128
NK = S // P          # 8 k-tiles of 128
NQ = S // 512        # 2 q-chunks of 512
# gm band-tile offsets (off = qc*512 - kt*128); first 4 also carry the mask.
# GM_W[i]: columns [0, w) of the tile that can hold nonzero gauss/mask values;
# outside the window gm == 0 so all gm work (ACT gen, msk add, identity-MM)
# runs on the window slice only.
OFFS = [0, -128, -256, -384, 128]
GM_W = [256, 384, 512, 512, 128]
F32 = mybir.dt.float32
BF16 = mybir.dt.bfloat16
AF = mybir.ActivationFunctionType


def _kts_for_qc(qc):
    # causal [k,q] tiles: tile has any unmasked cell iff kt*128 <= qc*512+511
    return list(range(4)) if qc == 0 else list(range(8))


def _gm_idx(kt, qc):
    off = qc * 512 - kt * 128
    return OFFS.index(off) if off in OFFS else None


def build_program():
    nc = bacc.Bacc("TRN2")

    xT_d = nc.dram_tensor("xT", [D, S], BF16, kind="ExternalInput")
    kT_d = nc.dram_tensor("kT", [D, S], BF16, kind="ExternalInput")
    vT_d = nc.dram_tensor("vT", [D, S], BF16, kind="ExternalInput")
    xres_d = nc.dram_tensor("xres", [S, D], F32, kind="ExternalInput")
    wq_d = nc.dram_tensor("wqT", [D, H * DK], BF16, kind="ExternalInput")
    wk_d = nc.dram_tensor("wkT", [D, H * DK], BF16, kind="ExternalInput")
    wv_d = nc.dram_tensor("wvT", [D, H * DV], BF16, kind="ExternalInput")
    wfc_d = nc.dram_tensor("wfcT", [H * DV, D], BF16, kind="ExternalInput")
    ga_d = nc.dram_tensor("ga", [len(OFFS), P, 512], F32, kind="ExternalInput")
    ms_d = nc.dram_tensor("ms", [4, P, 512], BF16, kind="ExternalInput")
    eye_d = nc.dram_tensor("eye", [P, P], BF16, kind="ExternalInput")

    attn_d = nc.dram_tensor("attnT", [H, S, S], BF16, kind="ExternalOutput")
    out_d = nc.dram_tensor("out", [S, D], F32, kind="ExternalOutput")

    with tile.TileContext(nc) as tc:
        with (
            tc.tile_pool(name="singles", bufs=1) as singles,
            tc.tile_pool(name="wtiles", bufs=2) as wtiles,
            tc.tile_pool(name="proj", bufs=2) as proj,
            tc.tile_pool(name="gmp", bufs=10) as gmp,
            tc.tile_pool(name="expp", bufs=3) as expp,
            tc.tile_pool(name="smallp", bufs=3) as smallp,
            tc.tile_pool(name="lnp", bufs=2) as lnp,
            tc.tile_pool(name="psum", bufs=2, space="PSUM") as psum,
            tc.tile_pool(name="psum_sT", bufs=3, space="PSUM") as psum_sT,
            tc.tile_pool(name="psum_ctx", bufs=3, space="PSUM") as psum_ctx,
        ):
            # ---------------- phase A: resident loads + V projection ----
            xT_sb = singles.tile([P, NK, S], BF16, tag="xT")
            kT_sb = singles.tile([P, NK, S], BF16, tag="kT")
            wv_sb = singles.tile([P, NK, H * DV], BF16, tag="wv")
            wfc_sb = singles.tile([P, NK, D], BF16, tag="wfc")
            ga_sb = singles.tile([P, len(OFFS), 512], F32, tag="ga")
            ms_sb = singles.tile([P, 4, 512], BF16, tag="ms")
            eye_sb = singles.tile([P, P], BF16, tag="eye")
            eps_sb = singles.tile([P, 1], F32, tag="eps")
            # V with a ones column appended per head: [.., h*65+64] == 1.0
            vaug_sb = singles.tile([P, NK, H * 65], BF16, tag="vaug")

            nc.sync.dma_start(xT_sb, xT_d.ap().rearrange("(o p) s -> p o s", p=P))
            nc.sync.dma_start(kT_sb, kT_d.ap().rearrange("(o p) s -> p o s", p=P))
            nc.sync.dma_start(wv_sb, wv_d.ap().rearrange("(o p) n -> p o n", p=P))
            nc.sync.dma_start(wfc_sb, wfc_d.ap().rearrange("(o p) n -> p o n", p=P))
            nc.sync.dma_start(ga_sb, ga_d.ap().rearrange("o p f -> p o f"))
            nc.sync.dma_start(ms_sb, ms_d.ap().rearrange("o p f -> p o f"))
            nc.sync.dma_start(eye_sb, eye_d.ap())
            nc.vector.memset(eps_sb, EPS)
            vaug4 = vaug_sb.rearrange("p o (h c) -> p o h c", c=65)
            nc.vector.memset(vaug4[:, :, :, 64:65], 1.0)

            # V = input_V @ Wv.T in normal [s, hv] layout
            for st in range(NK):
                ps = [psum.tile([P, 512], F32, tag="proj", name=f"vps{dc}")
                      for dc in range(2)]
                vt = wtiles.tile([P, NK, P], BF16, tag="vt")
                nc.sync.dma_start(
                    vt, vT_d.ap()[:, st * P:(st + 1) * P].rearrange(
                        "(o p) s -> p o s", p=P))
                for k in range(NK):
                    for dc in range(2):
                        nc.tensor.matmul(
                            ps[dc], vt[:, k, :],
                            wv_sb[:, k, dc * 512:(dc + 1) * 512],
                            start=(k == 0), stop=(k == NK - 1))
                for dc in range(2):
                    nc.vector.tensor_copy(
                        out=vaug4[:, st, dc * 8:(dc + 1) * 8, 0:64],
                        in_=ps[dc].rearrange("p (h c) -> p h c", c=64))

            # ctxT accumulator [hv%128, hv//128, q]
            ctxT_sb = singles.tile([P, NK, S], BF16, tag="ctxT")

            # ---------------- phase B: per head-pair ----------------
            for j in range(8):
                hA, hB = 2 * j, 2 * j + 1
                # QT_j / KT_j: [hk%128, s] for hk in [128j, 128j+128)
                qkt = {}
                for nm, w_d, rhs_sb in (("q", wq_d, xT_sb), ("k", wk_d, kT_sb)):
                    dst = proj.tile([P, S], BF16, tag=f"{nm}tj")
                    wt = wtiles.tile([P, NK, P], BF16, tag=f"w{nm}")
                    nc.sync.dma_start(
                        wt, w_d.ap()[:, j * P:(j + 1) * P].rearrange(
                            "(o p) n -> p o n", p=P))
                    for qc in range(2):
                        ps = psum.tile([P, 512], F32, tag="proj")
                        for k in range(NK):
                            nc.tensor.matmul(
                                ps, wt[:, k, :],
                                rhs_sb[:, k, qc * 512:(qc + 1) * 512],
                                start=(k == 0), stop=(k == NK - 1))
                        nc.vector.tensor_copy(
                            out=dst[:, qc * 512:(qc + 1) * 512], in_=ps)
                    qkt[nm] = dst

                # per-head gm band tiles: gauss(+mask)
                gms = {}
                for h in (hA, hB):
                    c_h = 1.0 / (2.0 * float(h + 1) ** 2)
                    for i in range(len(OFFS)):
                        w = GM_W[i]
                        g = gmp.tile([P, w], BF16, tag="gm")
                        nc.scalar.activation(
                            out=g, in_=ga_sb[:, i, :w], func=AF.Exp,
                            scale=-c_h)
                        if i < 4:
                            nc.vector.tensor_add(
                                out=g, in0=g, in1=ms_sb[:, i, :w])
                        gms[(h, i)] = g

                for qc in range(2):
                    kts = _kts_for_qc(qc)
                    exps = {h: expp.tile([P, NK, 512], BF16, tag="expST",
                                         name=f"expST{h % 2}")
                            for h in (hA, hB)}
                    ctxps = {h: psum_ctx.tile([65, 512], F32, tag="ctx",
                                              name=f"ctx{h % 2}")
                             for h in (hA, hB)}

                    def av_mms(kt, first, last):
                        for h in (hA, hB):
                            nc.tensor.matmul(
                                ctxps[h],
                                vaug_sb[:, kt, h * 65:(h + 1) * 65],
                                exps[h][:, kt, :],
                                start=first, stop=last,
                                skip_group_check=True)

                    for kt in kts:
                        gi = _gm_idx(kt, qc)
                        sTs = {}
                        # both heads' score MMs back to back: K=64 row groups
                        # 0-1 / 2-3 run concurrently on the PE
                        for h in (hA, hB):
                            hp = 64 * (h % 2)
                            sT = psum_sT.tile([P, 512], F32, tag="sT",
                                              name=f"sT{h % 2}")
                            nc.tensor.matmul(
                                sT,
                                qkt["k"][hp:hp + 64, kt * P:(kt + 1) * P],
                                qkt["q"][hp:hp + 64, qc * 512:(qc + 1) * 512],
                                start=True, stop=(gi is None),
                                skip_group_check=True)
                            sTs[h] = sT
                        if gi is not None:
                            for h in (hA, hB):
                                nc.tensor.matmul(
                                    sTs[h][:, :GM_W[gi]], eye_sb,
                                    gms[(h, gi)],
                                    start=False, stop=True,
                                    skip_group_check=True)
                        for h in (hA, hB):
                            nc.scalar.activation(
                                out=exps[h][:, kt, :], in_=sTs[h],
                                func=AF.Exp)
                        # lookahead: AV consumes the PREVIOUS kt's exps so the
                        # PE never waits on ACT
                        if kt != kts[0]:
                            av_mms(kt - 1, kt - 1 == kts[0], False)
                    av_mms(kts[-1], len(kts) == 1, True)
                    for h in (hA, hB):
                        hp = 64 * (h % 2)
                        nkt = len(kts)
                        # 1/rowsum via ACT: exp(-ln(sum)); ~1e-5 rel err and
                        # keeps the costly single-partition divide off DVE
                        ln = smallp.tile([1, 512], F32, tag="lnS")
                        nc.scalar.activation(
                            out=ln, in_=ctxps[h][64:65, :], func=AF.Ln)
                        rn = smallp.tile([1, 512], BF16, tag="recipN")
                        nc.scalar.activation(
                            out=rn, in_=ln, func=AF.Exp, scale=-1.0)
                        rb = smallp.tile([P, 512], BF16, tag="recipB")
                        nc.gpsimd.partition_broadcast(rb, rn)
                        nc.vector.tensor_mul(
                            out=ctxT_sb[hp:hp + 64, j, qc * 512:(qc + 1) * 512],
                            in0=ctxps[h][0:64, :], in1=rb[hp:hp + 64, :])
                        rb_b = bass.AP(
                            tensor=rb.tensor, offset=rb.offset,
                            ap=[list(rb.ap[0]), [0, nkt], list(rb.ap[1])])
                        nc.vector.tensor_mul(
                            out=exps[h][:, 0:nkt, :],
                            in0=exps[h][:, 0:nkt, :], in1=rb_b)
                        nc.gpsimd.dma_start(
                            out=attn_d.ap()[h].rearrange(
                                "(kt p) q -> p kt q",
                                p=P)[:, 0:nkt, qc * 512:(qc + 1) * 512],
                            in_=exps[h][:, 0:nkt, :])

            # ---------------- phase C: fc + residual + LayerNorm --------
            for st in range(NK):
                ps = [psum.tile([P, 512], F32, tag="proj", name=f"fcps{dc}")
                      for dc in range(2)]
                for dc in range(2):
                    for jj in range(NK):
                        nc.tensor.matmul(
                            ps[dc],
                            ctxT_sb[:, jj, st * P:(st + 1) * P],
                            wfc_sb[:, jj, dc * 512:(dc + 1) * 512],
                            start=(jj == 0), stop=(jj == NK - 1))
                xr = lnp.tile([P, D], F32, tag="xr")
                nc.sync.dma_start(xr, xres_d.ap()[st * P:(st + 1) * P, :])
                x2 = lnp.tile([P, D], F32, tag="x2")
                for dc in range(2):
                    nc.vector.tensor_add(
                        out=x2[:, dc * 512:(dc + 1) * 512], in0=ps[dc],
                        in1=xr[:, dc * 512:(dc + 1) * 512])
                stats = lnp.tile([P, 2, nc.vector.BN_STATS_DIM], F32, tag="bs")
                for g in range(2):
                    nc.vector.bn_stats(
                        out=stats[:, g, :], in_=x2[:, g * 512:(g + 1) * 512])
                mv = lnp.tile([P, nc.vector.BN_AGGR_DIM], F32, tag="mv")
                nc.vector.bn_aggr(out=mv, in_=stats)
                sd = lnp.tile([P, 1], F32, tag="sd")
                nc.scalar.activation(
                    out=sd, in_=mv[:, 1:2], func=AF.Sqrt, bias=eps_sb)
                nc.vector.reciprocal(out=sd, in_=sd)
                nc.vector.tensor_scalar(
                    out=x2, in0=x2, scalar1=mv[:, 0:1], scalar2=sd,
                    op0=mybir.AluOpType.subtract, op1=mybir.AluOpType.mult)
                nc.sync.dma_start(out=out_d.ap()[st * P:(st + 1) * P, :], in_=x2)

    nc.finalize()
    return nc


_prog_cache = {}


def _get_prog():
    if "nc" not in _prog_cache:
        _prog_cache["nc"] = build_program()
    return _prog_cache["nc"]


def _host_tables():
    ga = np.full((len(OFFS), P, 512), 1e13, dtype=np.float32)
    ms = np.zeros((4, P, 512), dtype=np.float32)
    pp = np.arange(P)[:, None]
    ff = np.arange(512)[None, :]
    for i, off in enumerate(OFFS):
        d = off + ff - pp
        band = (d >= 0) & (d <= 127)
        ga[i][band] = (d * d)[band].astype(np.float32)
        if i < 4:
            ms[i][d < 0] = -1e9
    return ga, ms.astype(ml_dtypes.bfloat16)


def _numpy_fallback(input_Q, input_K, input_V, attn_mask, Wq, Wk, Wv, Wfc):
    # generic-mask reference path (only used if attn_mask is not causal)
    b, s, d = input_Q.shape
    Q = (input_Q @ Wq.T).reshape(b, s, H, DK).transpose(0, 2, 1, 3)
    K = (input_K @ Wk.T).reshape(b, s, H, DK).transpose(0, 2, 1, 3)
    V = (input_V @ Wv.T).reshape(b, s, H, DV).transpose(0, 2, 1, 3)
    scores = np.einsum("bhqd,bhkd->bhqk", Q, K) / np.sqrt(np.float32(DK))
    i = np.arange(s)[:, None]
    jj = np.arange(s)[None, :]
    dist2 = ((i - jj).astype(np.float32)) ** 2
    sigma = np.arange(1, H + 1, dtype=np.float32)
    bias = np.exp(-dist2[None] / (2.0 * sigma[:, None, None] ** 2))
    bias = np.where(i >= jj, bias, 0.0)
    scores = scores + bias[None]
    scores = np.where(attn_mask[:, None], np.float32(-1e9), scores)
    scores -= scores.max(axis=-1, keepdims=True)
    e = np.exp(scores)
    attn = e / e.sum(axis=-1, keepdims=True)
    context = np.einsum("bhqk,bhkd->bhqd", attn, V)
    context = context.transpose(0, 2, 1, 3).reshape(b, s, H * DV)
    output = context @ Wfc.T
    x = output + input_Q
    mu = x.mean(axis=-1, keepdims=True)
    var = x.var(axis=-1, keepdims=True)
    out = (x - mu) / np.sqrt(var + EPS)
    return out.astype(np.float32), attn.astype(np.float32)


def kernel(input_Q, input_K, input_V, attn_mask, Wq, Wk, Wv, Wfc):
    input_Q = np.asarray(input_Q, dtype=np.float32)
    input_K = np.asarray(input_K, dtype=np.float32)
    input_V = np.asarray(input_V, dtype=np.float32)
    attn_mask = np.asarray(attn_mask)
    causal = np.triu(np.ones((S, S), dtype=bool), k=1)
    if not np.array_equal(attn_mask, np.broadcast_to(causal, (B, S, S))):
        return _numpy_fallback(input_Q, input_K, input_V, attn_mask,
                               np.asarray(Wq, np.float32),
                               np.asarray(Wk, np.float32),
                               np.asarray(Wv, np.float32),
                               np.asarray(Wfc, np.float32))

    bf = ml_dtypes.bfloat16
    wqT = (np.asarray(Wq, np.float32).T / np.sqrt(np.float32(DK))).astype(bf)
    wkT = np.asarray(Wk, np.float32).T.astype(bf)
    wvT = np.asarray(Wv, np.float32).T.astype(bf)
    wfcT = np.asarray(Wfc, np.float32).T.astype(bf)
    ga, ms = _host_tables()
    eye = np.eye(P, dtype=np.float32).astype(bf)

    in_maps = []
    for b in range(B):
        in_maps.append({
            "xT": np.ascontiguousarray(input_Q[b].T).astype(bf),
            "kT": np.ascontiguousarray(input_K[b].T).astype(bf),
            "vT": np.ascontiguousarray(input_V[b].T).astype(bf),
            "xres": np.ascontiguousarray(input_Q[b]),
            "wqT": wqT, "wkT": wkT, "wvT": wvT, "wfcT": wfcT,
            "ga": ga, "ms": ms, "eye": eye,
        })

    global _last_in_maps
    _last_in_maps = in_maps
    nc = _get_prog()
    res = run_bass_kernel_spmd(nc, in_maps, core_ids=list(range(B)))

    out = np.empty((B, S, D), dtype=np.float32)
    attn = np.empty((B, H, S, S), dtype=np.float32)
    for b in range(B):
        out[b] = res.results[b]["out"]
        attn[b] = res.results[b]["attnT"].astype(np.float32).transpose(0, 2, 1)
    return out, attn
